# revision 5
# baseline (speedup 1.0000x reference)
"""Trainium2 Bass kernel for nn_Dense_56779467653682.

Computes out = scale * x @ (2*kernel - 1) where x:[8,2048,4096] f32,
kernel:[4096,4096] bool, scale scalar f32 (= 1/64).

Strategy: data-parallel over the 16384 tokens across 8 NeuronCores
(2048 tokens/core). The ternary weight (+-scale, exact in bf16 since
scale is a power of two) is folded on the host into a bf16 weight
matrix, and x is cast to bf16 and pre-transposed/tiled on the host so
the device kernel is a pure dense matmul:

    per core: out[2048, 4096] f32 = x_bf16[2048, 4096] @ w_bf16[4096, 4096]

Device tiling (per core):
  - contraction K=4096 -> 32 k-tiles of 128 (partition dim)
  - tokens M=2048 -> 16 m-tiles of 128 (PSUM partition dim, lhsT free dim)
  - features N=4096 -> 8 n-chunks of 512 (PSUM free dim = one bank)
  All 16 xT m-tiles stay resident in SBUF (128 KB/partition); w streams
  once in 4 MB n-chunks (double buffered); each output tile accumulates
  32 back-to-back matmuls in one PSUM bank, is copied to SBUF on the
  DVE, and DMA'd out.
"""

import numpy as np
import ml_dtypes

BATCH, SEQ, IN_DIM, FEATURES = 8, 2048, 4096, 4096
N_CORES = 8
TOKENS = BATCH * SEQ
TOK_PER_CORE = TOKENS // N_CORES  # 2048
P = 128                           # partitions / tile edge
KT = IN_DIM // P                  # 32 k-tiles
MT = TOK_PER_CORE // P            # 16 m-tiles
NF = 512                          # features per n-chunk (one PSUM bank of f32)
NT = FEATURES // NF               # 8 n-chunks

_BF16 = ml_dtypes.bfloat16

_cache = {}


def _build_program():
    """Build + compile the per-core Bass/Tile program (SPMD, same on all cores)."""
    import concourse.bacc as bacc
    import concourse.mybir as mybir
    from concourse.tile import TileContext

    nc = bacc.Bacc("TRN2", target_bir_lowering=False, debug=False)

    xs_d = nc.dram_tensor("xs", [MT, P, KT, P], mybir.dt.bfloat16, kind="ExternalInput")
    ws_d = nc.dram_tensor("ws", [NT, P, KT, NF], mybir.dt.bfloat16, kind="ExternalInput")
    out_d = nc.dram_tensor("out", [TOK_PER_CORE, FEATURES], mybir.dt.float32, kind="ExternalOutput")

    with TileContext(nc) as tc:
        with (
            tc.tile_pool(name="xpool", bufs=1) as xpool,
            tc.tile_pool(name="wpool", bufs=2) as wpool,
            tc.tile_pool(name="epool", bufs=4) as epool,
            tc.tile_pool(name="psum", bufs=4, space="PSUM") as pp,
        ):
            # Resident xT tiles: [k-partition, k-tile, token] per m-tile.
            # First m-tile's DMA goes first so the PE can start ASAP; the
            # first w chunk follows it, then the rest of x streams in.
            xs_t = []
            x0 = xpool.tile([P, KT, P], mybir.dt.bfloat16, name="xs_t0")
            nc.sync.dma_start(out=x0[:], in_=xs_d[0])
            xs_t.append(x0)

            w_tiles = [None] * NT
            w0 = wpool.tile([P, KT, NF], mybir.dt.bfloat16, name="w_t0", tag="w")
            nc.sync.dma_start(out=w0[:], in_=ws_d[0])
            w_tiles[0] = w0

            for mt in range(1, MT):
                xt = xpool.tile([P, KT, P], mybir.dt.bfloat16, name=f"xs_t{mt}")
                nc.sync.dma_start(out=xt[:], in_=xs_d[mt])
                xs_t.append(xt)

            for nt in range(NT):
                if w_tiles[nt] is None:
                    wt = wpool.tile([P, KT, NF], mybir.dt.bfloat16, name=f"w_t{nt}", tag="w")
                    nc.sync.dma_start(out=wt[:], in_=ws_d[nt])
                    w_tiles[nt] = wt
                wt = w_tiles[nt]
                for mt in range(MT):
                    ps = pp.tile([P, NF], mybir.dt.float32, name="ps", tag="ps")
                    for ko in range(KT):
                        nc.tensor.matmul(
                            ps[:],
                            xs_t[mt][:, ko, :],
                            wt[:, ko, :],
                            start=(ko == 0),
                            stop=(ko == KT - 1),
                        )
                    ev = epool.tile([P, NF], mybir.dt.float32, name="ev", tag="ev")
                    nc.vector.tensor_copy(ev[:], ps[:])
                    nc.sync.dma_start(
                        out=out_d[mt * P:(mt + 1) * P, nt * NF:(nt + 1) * NF],
                        in_=ev[:],
                    )

    nc.compile()
    return nc


def _prep_inputs(x, kern, scale):
    """Host-side: fold scale into ternary bf16 weights; cast+tile x per core."""
    s = float(np.asarray(scale))
    # w[k, f] = +-scale, exact in bf16 when scale is a power of two.
    w = np.where(np.asarray(kern), np.float32(s), np.float32(-s)).astype(_BF16)
    # ws[nt, kp, ko, n] = w[ko*128 + kp, nt*512 + n]
    ws = np.ascontiguousarray(
        w.reshape(KT, P, NT, NF).transpose(2, 1, 0, 3)
    )

    xf = np.asarray(x).reshape(TOKENS, IN_DIM).astype(_BF16)
    in_maps = []
    for c in range(N_CORES):
        xc = xf[c * TOK_PER_CORE:(c + 1) * TOK_PER_CORE]
        # xs[mt, kp, ko, mi] = xc[mt*128 + mi, ko*128 + kp]
        xs = np.ascontiguousarray(
            xc.reshape(MT, P, KT, P).transpose(0, 3, 2, 1)
        )
        in_maps.append({"xs": xs, "ws": ws})
    return in_maps


def _run(inputs, trace=False, tmpdir=None):
    from concourse.bass_utils import run_bass_kernel_spmd

    if "nc" not in _cache:
        _cache["nc"] = _build_program()
    nc = _cache["nc"]

    in_maps = _prep_inputs(inputs["x"], inputs["kernel"], inputs["scale"])
    res = run_bass_kernel_spmd(
        nc, in_maps, core_ids=list(range(N_CORES)), trace=trace, tmpdir=tmpdir
    )
    out = np.concatenate(
        [res.results[c]["out"][None] for c in range(N_CORES)], axis=0
    ).reshape(BATCH, SEQ, FEATURES)
    return np.ascontiguousarray(out.astype(np.float32, copy=False)), res


def kernel(**inputs):
    out, _ = _run(inputs, trace=False)
    return out


# revision 6
# speedup vs baseline: 1.0066x; 1.0066x over previous
"""Trainium2 Bass kernel for nn_Dense_56779467653682.

Computes out = scale * x @ (2*kernel - 1) where x:[8,2048,4096] f32,
kernel:[4096,4096] bool, scale scalar f32 (= 1/64).

Strategy: data-parallel over the 16384 tokens across 8 NeuronCores
(2048 tokens/core). The ternary weight (+-scale, exact in bf16 since
scale is a power of two) is folded on the host into a bf16 weight
matrix, and x is cast to bf16 and pre-transposed/tiled on the host so
the device kernel is a pure dense matmul:

    per core: out[2048, 4096] f32 = x_bf16[2048, 4096] @ w_bf16[4096, 4096]

Device tiling (per core):
  - contraction K=4096 -> 32 k-tiles of 128 (partition dim)
  - tokens M=2048 -> 16 m-tiles of 128 (PSUM partition dim, lhsT free dim)
  - features N=4096 -> 8 n-chunks of 512 (PSUM free dim = one bank)
  All 16 xT m-tiles stay resident in SBUF (128 KB/partition); w streams
  once in 4 MB n-chunks (double buffered); each output tile accumulates
  32 back-to-back matmuls in one PSUM bank, is copied to SBUF on the
  DVE, and DMA'd out.
"""

import numpy as np
import ml_dtypes

BATCH, SEQ, IN_DIM, FEATURES = 8, 2048, 4096, 4096
N_CORES = 8
TOKENS = BATCH * SEQ
TOK_PER_CORE = TOKENS // N_CORES  # 2048
P = 128                           # partitions / tile edge
KT = IN_DIM // P                  # 32 k-tiles
MT = TOK_PER_CORE // P            # 16 m-tiles
NF = 512                          # features per n-chunk (one PSUM bank of f32)
NT = FEATURES // NF               # 8 n-chunks

_BF16 = ml_dtypes.bfloat16

_cache = {}


def _build_program():
    """Build + compile the per-core Bass/Tile program (SPMD, same on all cores)."""
    import concourse.bacc as bacc
    import concourse.mybir as mybir
    from concourse.tile import TileContext

    nc = bacc.Bacc("TRN2", target_bir_lowering=False, debug=False)

    xs_d = nc.dram_tensor("xs", [MT, P, KT, P], mybir.dt.bfloat16, kind="ExternalInput")
    ws_d = nc.dram_tensor("ws", [NT, P, KT, NF], mybir.dt.bfloat16, kind="ExternalInput")
    out_d = nc.dram_tensor("out", [TOK_PER_CORE, FEATURES], mybir.dt.float32, kind="ExternalOutput")

    KG = 8                 # k-tiles per w sub-tile (fine-grained RAW deps)
    NSUB = KT // KG        # 4 sub-tiles per n-chunk
    WARMUP_MMS = 30        # dummy matmuls to lift HAM to K=8/8 during input DMA

    with TileContext(nc) as tc:
        with (
            tc.tile_pool(name="xpool", bufs=1) as xpool,
            tc.tile_pool(name="wpool", bufs=2 * NSUB) as wpool,
            tc.tile_pool(name="epool", bufs=4) as epool,
            tc.tile_pool(name="warm", bufs=1) as warm,
            tc.tile_pool(name="psum", bufs=4, space="PSUM") as pp,
            tc.tile_pool(name="psumw", bufs=1, space="PSUM") as ppw,
        ):
            # PE warmup: the HAM clock gate only reaches 2.4 GHz after ~3.4us
            # of sustained PE activity. Burn the initial DMA wait on dummy
            # matmuls so the real ones start at full clock.
            wu = warm.tile([P, 256], mybir.dt.bfloat16, name="wu")
            nc.gpsimd.memset(wu[:], 0.0)
            wups = ppw.tile([P, 256], mybir.dt.float32, name="wups")
            for _ in range(WARMUP_MMS):
                nc.tensor.matmul(wups[:], wu[:, :P], wu[:], start=True, stop=True)

            # Resident xT tiles: [k-partition, k-tile, token] per m-tile.
            # First m-tile's DMA goes first so the PE can start ASAP; the
            # first w sub-chunk follows it, then the rest of x streams in.
            xs_t = []
            x0 = xpool.tile([P, KT, P], mybir.dt.bfloat16, name="xs_t0")
            nc.sync.dma_start(out=x0[:], in_=xs_d[0])
            xs_t.append(x0)

            # w is streamed as [128, KG, 512] sub-tiles so the first matmuls
            # only wait on a 1 MB DMA, not the full 4 MB chunk.
            w_tiles = [None] * NT

            def load_w(nt):
                subs = []
                for g in range(NSUB):
                    wt = wpool.tile(
                        [P, KG, NF], mybir.dt.bfloat16, name=f"w_{nt}_{g}", tag="w"
                    )
                    nc.sync.dma_start(
                        out=wt[:], in_=ws_d[nt, :, g * KG:(g + 1) * KG, :]
                    )
                    subs.append(wt)
                w_tiles[nt] = subs

            load_w(0)

            for mt in range(1, MT):
                xt = xpool.tile([P, KT, P], mybir.dt.bfloat16, name=f"xs_t{mt}")
                nc.sync.dma_start(out=xt[:], in_=xs_d[mt])
                xs_t.append(xt)

            for nt in range(NT):
                if w_tiles[nt] is None:
                    load_w(nt)
                wt = w_tiles[nt]
                for mt in range(MT):
                    ps = pp.tile([P, NF], mybir.dt.float32, name="ps", tag="ps")
                    for ko in range(KT):
                        nc.tensor.matmul(
                            ps[:],
                            xs_t[mt][:, ko, :],
                            wt[ko // KG][:, ko % KG, :],
                            start=(ko == 0),
                            stop=(ko == KT - 1),
                        )
                    ev = epool.tile([P, NF], mybir.dt.float32, name="ev", tag="ev")
                    nc.vector.tensor_copy(ev[:], ps[:])
                    nc.sync.dma_start(
                        out=out_d[mt * P:(mt + 1) * P, nt * NF:(nt + 1) * NF],
                        in_=ev[:],
                    )

    nc.compile()
    return nc


def _prep_inputs(x, kern, scale):
    """Host-side: fold scale into ternary bf16 weights; cast+tile x per core."""
    s = float(np.asarray(scale))
    # w[k, f] = +-scale, exact in bf16 when scale is a power of two.
    w = np.where(np.asarray(kern), np.float32(s), np.float32(-s)).astype(_BF16)
    # ws[nt, kp, ko, n] = w[ko*128 + kp, nt*512 + n]
    ws = np.ascontiguousarray(
        w.reshape(KT, P, NT, NF).transpose(2, 1, 0, 3)
    )

    xf = np.asarray(x).reshape(TOKENS, IN_DIM).astype(_BF16)
    in_maps = []
    for c in range(N_CORES):
        xc = xf[c * TOK_PER_CORE:(c + 1) * TOK_PER_CORE]
        # xs[mt, kp, ko, mi] = xc[mt*128 + mi, ko*128 + kp]
        xs = np.ascontiguousarray(
            xc.reshape(MT, P, KT, P).transpose(0, 3, 2, 1)
        )
        in_maps.append({"xs": xs, "ws": ws})
    return in_maps


def _run(inputs, trace=False, tmpdir=None):
    from concourse.bass_utils import run_bass_kernel_spmd

    if "nc" not in _cache:
        _cache["nc"] = _build_program()
    nc = _cache["nc"]

    in_maps = _prep_inputs(inputs["x"], inputs["kernel"], inputs["scale"])
    res = run_bass_kernel_spmd(
        nc, in_maps, core_ids=list(range(N_CORES)), trace=trace, tmpdir=tmpdir
    )
    out = np.concatenate(
        [res.results[c]["out"][None] for c in range(N_CORES)], axis=0
    ).reshape(BATCH, SEQ, FEATURES)
    return np.ascontiguousarray(out.astype(np.float32, copy=False)), res


def kernel(**inputs):
    out, _ = _run(inputs, trace=False)
    return out
